# revision 1
# baseline (speedup 1.0000x reference)
"""Grouped Conv2D (32 groups of 8->8 ch, 3x3, SAME) on 8 trn2 NeuronCores.

Strategy:
  - Data-parallel over batch: 32 images / 8 cores = 4 images per core.
  - Grouped conv as implicit GEMM: for each of the 9 taps, a 128x128
    block-diagonal stationary (16 groups x [8ic x 8oc] blocks) multiplies a
    tap-shifted view of the zero-haloed input image, accumulating in PSUM.
  - bf16 inputs (host-cast) / bf16 weights, fp32 PSUM accumulate.
  - Per-core: 4 images x 2 channel-chunks x 7 row-strips x 9 taps matmuls.
"""

import sys

import numpy as np

if "/opt/trn_rl_repo" not in sys.path:
    sys.path.insert(0, "/opt/trn_rl_repo")

import ml_dtypes

B, C, H, W = 32, 256, 56, 56
KK = 3
GROUPS = 32
CPG = 8  # in- and out-channels per group
N_CORES = 8
BPC = B // N_CORES  # images per core
HP, WP = H + 2, W + 2  # padded image
NCHUNK = 2  # 256 channels = 2 x 128 partitions
GPC = 16  # groups per chunk
STRIP = 8  # output rows per PSUM strip (8*56=448 <= 512 fp32/bank)
NSTRIP = H // STRIP
# strips grouped into multi-bank psum tiles: (start_strip, n_strips)
PSUM_GROUPS = [(0, 4), (4, 3)]


def _pack_weights(w: np.ndarray) -> np.ndarray:
    """[256, 8, 3, 3] fp32 -> [128 pc, 2 chunk, 9 tap, 128 po] bf16 block-diag.

    lhsT[pc=8*gl+ic, po=8*gl+oc_local] = w[128*ck + 8*gl + oc_local, ic, th, tw]
    """
    wr = w.reshape(NCHUNK, GPC, CPG, CPG, KK, KK)  # ck, gl, o, ic, th, tw
    wpk = np.zeros((GPC, CPG, NCHUNK, KK * KK, GPC, CPG), dtype=np.float32)
    for gl in range(GPC):
        # [ck, o, ic, th, tw] -> [ic, ck, (th tw), o]
        blk = wr[:, gl].transpose(2, 0, 3, 4, 1).reshape(CPG, NCHUNK, KK * KK, CPG)
        wpk[gl, :, :, :, gl, :] = blk
    return wpk.reshape(128, NCHUNK, KK * KK, 128).astype(ml_dtypes.bfloat16)


def _build_bass():
    import concourse.tile as tile
    from concourse import bacc, mybir

    nc = bacc.Bacc()
    xs = nc.dram_tensor(
        "xs", [BPC, C, HP, WP], mybir.dt.bfloat16, kind="ExternalInput"
    )
    wpk = nc.dram_tensor(
        "wpk", [128, NCHUNK, KK * KK, 128], mybir.dt.bfloat16, kind="ExternalInput"
    )
    out = nc.dram_tensor("out", [BPC, C, H, W], mybir.dt.float32, kind="ExternalOutput")

    with tile.TileContext(nc) as tc:
        with (
            tc.tile_pool(name="singles", bufs=1) as singles,
            tc.tile_pool(name="xpad_pool", bufs=3) as xpad_pool,
            tc.tile_pool(name="ot_pool", bufs=4) as ot_pool,
            tc.tile_pool(name="psum_pool", bufs=2, space="PSUM") as psum_pool,
        ):
            # HW-DGE lane order: w, in0, out_img0..2, out_img3_ck0/ck1A/ck1B (8)
            # SW-DGE lane order: in1..in7 (7) -- zero lane reuse, so every DMA
            # carries at most one sync wait (walrus hard limit). xpad bufs=3
            # throttles later inputs via buffer-reuse waits (their only wait),
            # avoiding HBM contention with the critical first tiles.
            w_sb = singles.tile([128, NCHUNK, KK * KK, 128], mybir.dt.bfloat16)
            nc.sync.dma_start(out=w_sb[:], in_=wpk[:])

            # PE warm-up: data-independent matmuls on the weight tile so the
            # HAM clock-gate releases (1.2 -> 2.4 GHz) before real work lands.
            wu = psum_pool.tile([128, 4, 512], mybir.dt.float32, name="ps")
            for i in range(4):
                nc.tensor.matmul(
                    wu[:, i, :],
                    lhsT=w_sb[:, 0, 0, :],
                    rhs=w_sb[:, 0, 0:4, :],
                    start=True,
                    stop=True,
                )

            def do_chunk(xpad, ck, sink):
                """Matmul both psum groups of one (image, chunk); sink(group
                index, psum tile, ns) consumes each accumulated group."""
                for gi, (s0, ns) in enumerate(PSUM_GROUPS):
                    ps = psum_pool.tile([128, 4, 512], mybir.dt.float32, name="ps")
                    for t in range(KK * KK):
                        th, tw = divmod(t, KK)
                        for si in range(ns):
                            s = s0 + si
                            nc.tensor.matmul(
                                ps[:, si, : STRIP * W],
                                lhsT=w_sb[:, ck, t, :],
                                rhs=xpad[
                                    :,
                                    s * STRIP + th : s * STRIP + th + STRIP,
                                    tw : tw + W,
                                ],
                                start=(t == 0),
                                stop=(t == KK * KK - 1),
                            )
                    sink(gi, ps, s0, ns)

            # group-A matmuls of (0,0) only need padded rows 0..33; splitting
            # the first input at that boundary halves time-to-first-matmul
            # (subtile deps let group A start on the first half alone).
            SPLIT = PSUM_GROUPS[0][1] * STRIP + 2  # 34

            def load_xpad(b, ck):
                xpad = xpad_pool.tile([128, HP, WP], mybir.dt.bfloat16, name="xpad")
                if (b, ck) == (0, 0):
                    nc.sync.dma_start(
                        out=xpad[:, :SPLIT, :],
                        in_=xs[b, ck * 128 : (ck + 1) * 128, :SPLIT, :],
                    )
                    nc.gpsimd.dma_start(
                        out=xpad[:, SPLIT:, :],
                        in_=xs[b, ck * 128 : (ck + 1) * 128, SPLIT:, :],
                    )
                else:
                    nc.gpsimd.dma_start(
                        out=xpad[:], in_=xs[b, ck * 128 : (ck + 1) * 128]
                    )
                return xpad

            xpads = {(b, ck): load_xpad(b, ck) for b in range(BPC) for ck in range(NCHUNK)}

            for b in range(BPC - 1):
                ot = ot_pool.tile([128, NCHUNK, H, W], mybir.dt.float32, name="oti", bufs=3)
                for ck in range(NCHUNK):
                    def sink(gi, ps, s0, ns, _ot=ot, _ck=ck):
                        nc.scalar.copy(
                            out=_ot[:, _ck, s0 * STRIP : (s0 + ns) * STRIP, :],
                            in_=ps[:, :ns, : STRIP * W],
                        )
                    do_chunk(xpads[(b, ck)], ck, sink)
                dst = out[b].rearrange("(ck c) h w -> c ck h w", ck=NCHUNK)
                nc.sync.dma_start(out=dst, in_=ot[:])

            # last image: finer-grained drain so the tail exposes only the
            # final 24-row group (copied + DMA'd on the spare HW lane).
            b = BPC - 1
            ot3 = ot_pool.tile([128, H, W], mybir.dt.float32, name="ot3", bufs=1)
            def sink3a(gi, ps, s0, ns):
                nc.scalar.copy(
                    out=ot3[:, s0 * STRIP : (s0 + ns) * STRIP, :],
                    in_=ps[:, :ns, : STRIP * W],
                )
            do_chunk(xpads[(b, 0)], 0, sink3a)
            nc.sync.dma_start(out=out[b, 0:128], in_=ot3[:])

            def sink3b(gi, ps, s0, ns):
                otg = ot_pool.tile([128, 4 * STRIP, W], mybir.dt.float32, name="otg", bufs=2)
                nc.scalar.copy(
                    out=otg[:, : ns * STRIP, :], in_=ps[:, :ns, : STRIP * W]
                )
                nc.sync.dma_start(
                    out=out[b, 128:256, s0 * STRIP : (s0 + ns) * STRIP, :],
                    in_=otg[:, : ns * STRIP, :],
                )
            do_chunk(xpads[(b, 1)], 1, sink3b)
    nc.finalize()
    return nc


_CACHE = {}


def kernel(x, w, trace=False):
    from concourse.bass_utils import run_bass_kernel_spmd

    x = np.asarray(x)
    w = np.ascontiguousarray(np.asarray(w), dtype=np.float32)

    if "nc" not in _CACHE:
        _CACHE["nc"] = _build_bass()
    nc = _CACHE["nc"]

    xbf = np.zeros((B, C, HP, WP), dtype=ml_dtypes.bfloat16)
    xbf[:, :, 1 : H + 1, 1 : W + 1] = x.astype(ml_dtypes.bfloat16)
    wpk = _pack_weights(w)
    in_maps = [
        {"xs": np.ascontiguousarray(xbf[i * BPC : (i + 1) * BPC]), "wpk": wpk}
        for i in range(N_CORES)
    ]
    res = run_bass_kernel_spmd(
        nc, in_maps, core_ids=list(range(N_CORES)), trace=trace
    )
    out = np.concatenate([res.results[i]["out"] for i in range(N_CORES)], axis=0)
    if trace:
        kernel.last_result = res
    return out



# revision 3
# speedup vs baseline: 1.2465x; 1.2465x over previous
"""Grouped Conv2D (32 groups of 8->8 ch, 3x3, SAME) on 8 trn2 NeuronCores.

Strategy:
  - Data-parallel over batch: 32 images / 8 cores = 4 images per core.
  - 4x4 PE-array tiling (16 concurrent 32x32 tiles): 4 column-groups each
    stream a different (image, 8-row strip) instance; the 4 row-tiles of a
    column share its stream and hold block-diagonal weights for 4 groups
    (4 x [8ic x 8oc]), so stationary density is 25% vs 6.25% untiled.
  - Per batch of 4 instances: 9 taps x 16 tile-matmuls accumulate in PSUM
    (bank r, partitions 32c), then one engine copy (scalar/vector
    alternating) evacuates all 4 banks to bf16 SBUF, then 4 DMAs scatter
    to HBM (channel de-interleave done by the DMA access pattern).
  - bf16 in/out over HBM (host casts), fp32 PSUM accumulate.
"""

import sys

import numpy as np

if "/opt/trn_rl_repo" not in sys.path:
    sys.path.insert(0, "/opt/trn_rl_repo")

import ml_dtypes

B, C, H, W = 32, 256, 56, 56
KK = 3
GROUPS = 32
CPG = 8  # in- and out-channels per group
N_CORES = 8
BPC = B // N_CORES  # images per core
HP, WP = H + 2, W + 2  # padded image
NCHUNK = 2  # 256 channels = 2 x 128 partitions
STRIP = 8  # output rows per instance (8*56=448 <= 512 fp32/bank)
NSTRIP = H // STRIP  # 7
NTAP = KK * KK
NCOL = 4  # concurrent column-groups (instances)
NROW = 4  # row-tiles per column (4 groups each)
WU_ROUNDS = 6  # PE warm-up waves


def _pack_weights(w: np.ndarray) -> np.ndarray:
    """[256, 8, 3, 3] fp32 -> [128 pc, 2 ck, 9 tap, 32] bf16.

    wpk[32r + 8j + ic, ck, 3*th+tw, 8j + oc] = w[128ck + 32r + 8j + oc, ic, th, tw]
    """
    wr = w.reshape(NCHUNK, NROW, 4, CPG, CPG, KK, KK)  # ck, r, j, oc, ic, th, tw
    wpk = np.zeros((NROW, 4, CPG, NCHUNK, NTAP, 4, CPG), dtype=np.float32)
    for j in range(4):
        # [ck, r, oc, ic, th, tw] -> [r, ic, ck, (th tw), oc]
        blk = wr[:, :, j].transpose(1, 3, 0, 4, 5, 2).reshape(NROW, CPG, NCHUNK, NTAP, CPG)
        wpk[:, j, :, :, :, j, :] = blk
    return wpk.reshape(128, NCHUNK, NTAP, 32).astype(ml_dtypes.bfloat16)


def _build_bass():
    import concourse.tile as tile
    from concourse import bacc, mybir

    nc = bacc.Bacc()
    xs = nc.dram_tensor(
        "xs", [BPC, C, HP, WP], mybir.dt.bfloat16, kind="ExternalInput"
    )
    wpk = nc.dram_tensor(
        "wpk", [128, NCHUNK, NTAP, 32], mybir.dt.bfloat16, kind="ExternalInput"
    )
    out = nc.dram_tensor(
        "out", [BPC, C, H, W], mybir.dt.bfloat16, kind="ExternalOutput"
    )

    with tile.TileContext(nc) as tc:
        with (
            tc.tile_pool(name="singles", bufs=1) as singles,
            tc.tile_pool(name="xpad_pool", bufs=3) as xpad_pool,
            tc.tile_pool(name="ot_pool", bufs=3) as ot_pool,
            tc.tile_pool(name="psum_pool", bufs=2, space="PSUM") as psum_pool,
        ):
            w_sb = singles.tile([128, NCHUNK, NTAP, 32], mybir.dt.bfloat16)
            nc.sync.dma_start(out=w_sb[:], in_=wpk[:])

            # PE warm-up in the same 4x4 tiled mode as the real matmuls
            # (mode switches drain the array), on weight-tile junk data,
            # while the first input streams in.
            wu = psum_pool.tile([128, NROW, 512], mybir.dt.float32, name="ps")
            for _ in range(WU_ROUNDS):
                for cg in range(NCOL):
                    for r in range(NROW):
                        nc.tensor.matmul(
                            wu[32 * cg : 32 * cg + 32, r, : STRIP * W],
                            lhsT=w_sb[32 * r : 32 * r + 32, 0, 0, :],
                            rhs=w_sb[32 * r : 32 * r + 32, 0:2, 0:7, :],
                            start=True,
                            stop=True,
                            tile_position=(32 * r, 32 * cg),
                        )

            # input tiles, issued in consumption order (ck-major, img minor);
            # bufs=3 throttles SW-DGE so early tiles aren't starved by
            # round-robin across many queued DMAs. First tile split at the
            # row needed by batch 0 (strips 0-3 -> padded rows 0:34), first
            # half on HW-DGE for a fast path to the first matmul.
            SPLIT = NCOL * STRIP + 2  # 34
            xpads = {}
            for ck in range(NCHUNK):
                for img in range(BPC):
                    xp = xpad_pool.tile([128, HP, WP], mybir.dt.bfloat16, name="xpad")
                    src = xs[img, ck * 128 : (ck + 1) * 128]
                    if (ck, img) == (0, 0):
                        nc.sync.dma_start(out=xp[:, :SPLIT, :], in_=src[:, :SPLIT, :])
                        nc.gpsimd.dma_start(out=xp[:, SPLIT:, :], in_=src[:, SPLIT:, :])
                    else:
                        nc.gpsimd.dma_start(out=xp[:], in_=src)
                    xpads[(ck, img)] = xp

            # 56 instances = 2 chunks x (4 images x 7 strips); batches of 4
            n_batch = 0
            for ck in range(NCHUNK):
                insts = [(j // NSTRIP, j % NSTRIP) for j in range(BPC * NSTRIP)]
                for k in range(len(insts) // NCOL):
                    quad = insts[NCOL * k : NCOL * k + NCOL]
                    ps = psum_pool.tile([128, NROW, 512], mybir.dt.float32, name="ps")
                    for t in range(NTAP):
                        th, tw = divmod(t, KK)
                        for cg, (img, s) in enumerate(quad):
                            for r in range(NROW):
                                nc.tensor.matmul(
                                    ps[32 * cg : 32 * cg + 32, r, : STRIP * W],
                                    lhsT=w_sb[32 * r : 32 * r + 32, ck, t, :],
                                    rhs=xpads[(ck, img)][
                                        32 * r : 32 * r + 32,
                                        s * STRIP + th : s * STRIP + th + STRIP,
                                        tw : tw + W,
                                    ],
                                    start=(t == 0),
                                    stop=(t == NTAP - 1),
                                    tile_position=(32 * r, 32 * cg),
                                )
                    ot = ot_pool.tile(
                        [128, NROW, STRIP, W], mybir.dt.bfloat16, name="ot"
                    )
                    copy = nc.scalar.copy if n_batch % 2 == 0 else nc.vector.tensor_copy
                    copy(out=ot[:], in_=ps[:, :, : STRIP * W])
                    for cg, (img, s) in enumerate(quad):
                        dst = out[
                            img, ck * 128 : (ck + 1) * 128, s * STRIP : (s + 1) * STRIP, :
                        ].rearrange("(r p) h w -> p r h w", r=NROW)
                        nc.sync.dma_start(
                            out=dst, in_=ot[32 * cg : 32 * cg + 32]
                        )
                    n_batch += 1
    nc.finalize()
    return nc


_CACHE = {}


def kernel(x, w, trace=False):
    from concourse.bass_utils import run_bass_kernel_spmd

    x = np.asarray(x)
    w = np.ascontiguousarray(np.asarray(w), dtype=np.float32)

    if "nc" not in _CACHE:
        _CACHE["nc"] = _build_bass()
    nc = _CACHE["nc"]

    xbf = np.zeros((B, C, HP, WP), dtype=ml_dtypes.bfloat16)
    xbf[:, :, 1 : H + 1, 1 : W + 1] = x.astype(ml_dtypes.bfloat16)
    wpk = _pack_weights(w)
    in_maps = [
        {"xs": np.ascontiguousarray(xbf[i * BPC : (i + 1) * BPC]), "wpk": wpk}
        for i in range(N_CORES)
    ]
    res = run_bass_kernel_spmd(
        nc, in_maps, core_ids=list(range(N_CORES)), trace=trace
    )
    outs = np.concatenate([res.results[i]["out"] for i in range(N_CORES)], axis=0)
    if trace:
        kernel.last_result = res
    return outs.astype(np.float32)


# revision 5
# speedup vs baseline: 1.7035x; 1.3666x over previous
"""Grouped Conv2D (32 groups of 8->8 ch, 3x3, SAME) on 8 trn2 NeuronCores.

Strategy:
  - Data-parallel over batch: 32 images / 8 cores = 4 images per core.
  - 2x2 PE-array tiling (4 concurrent 64x64 tiles): 2 column-groups each
    stream a different (image, 8-row strip) instance; the 2 row-tiles of a
    column share its stream and hold block-diagonal weights for 8 groups
    (8 x [8ic x 8oc]) -> 512 useful MACs per streamed instruction-column,
    the max for this group structure. ~8 matmuls in flight (4 tiles x 2
    pipelined) saturates the instruction streaming rate.
  - Per batch of 2 instances: 9 taps x 4 tile-matmuls accumulate in PSUM
    (bank r, partitions 64c), then one engine copy (scalar/vector
    alternating) evacuates both banks to bf16 SBUF, then 2 DMAs scatter
    to HBM (channel de-interleave via the DMA access pattern).
  - bf16 in/out over HBM (host casts), fp32 PSUM accumulate.
"""

import sys

import numpy as np

if "/opt/trn_rl_repo" not in sys.path:
    sys.path.insert(0, "/opt/trn_rl_repo")

import ml_dtypes

B, C, H, W = 32, 256, 56, 56
KK = 3
GROUPS = 32
CPG = 8  # in- and out-channels per group
N_CORES = 8
BPC = B // N_CORES  # images per core
HP, WP = H + 2, W + 2  # padded image
NCHUNK = 2  # 256 channels = 2 x 128 partitions
STRIP = 8  # output rows per instance (8*56=448 <= 512 fp32/bank)
NSTRIP = H // STRIP  # 7
NTAP = KK * KK
NCOL = 2  # concurrent column-groups (instances)
NROW = 2  # row-tiles per column (8 groups each)
GPT = 8  # groups per tile
WU_ROUNDS = 24  # PE warm-up waves (data-independent, start immediately)


def _pack_weights(w: np.ndarray) -> np.ndarray:
    """[256, 8, 3, 3] fp32 -> [128 pc, 2 ck, 9 tap, 64] bf16.

    wpk[64r + 8j + ic, ck, 3*th+tw, 8j + oc] = w[128ck + 64r + 8j + oc, ic, th, tw]
    """
    wr = w.reshape(NCHUNK, NROW, GPT, CPG, CPG, KK, KK)  # ck, r, j, oc, ic, th, tw
    wpk = np.zeros((NROW, GPT, CPG, NCHUNK, NTAP, GPT, CPG), dtype=np.float32)
    for j in range(GPT):
        # [ck, r, oc, ic, th, tw] -> [r, ic, ck, (th tw), oc]
        blk = wr[:, :, j].transpose(1, 3, 0, 4, 5, 2).reshape(NROW, CPG, NCHUNK, NTAP, CPG)
        wpk[:, j, :, :, :, j, :] = blk
    return wpk.reshape(128, NCHUNK, NTAP, 64).astype(ml_dtypes.bfloat16)


def _build_bass():
    import concourse.tile as tile
    from concourse import bacc, mybir

    nc = bacc.Bacc()
    xs = nc.dram_tensor(
        "xs", [BPC, C, HP, WP], mybir.dt.bfloat16, kind="ExternalInput"
    )
    wpk = nc.dram_tensor(
        "wpk", [128, NCHUNK, NTAP, 64], mybir.dt.bfloat16, kind="ExternalInput"
    )
    out = nc.dram_tensor(
        "out", [BPC, C, H, W], mybir.dt.bfloat16, kind="ExternalOutput"
    )

    with tile.TileContext(nc) as tc:
        with (
            tc.tile_pool(name="singles", bufs=1) as singles,
            tc.tile_pool(name="xpad_pool", bufs=3) as xpad_pool,
            tc.tile_pool(name="ot_pool", bufs=4) as ot_pool,
            tc.tile_pool(name="psum_pool", bufs=4, space="PSUM") as psum_pool,
        ):
            # PE warm-up in the same 2x2 tiled mode as the real matmuls
            # (mode switches drain the array), on a memset scratch tile so
            # it needs no input data and starts immediately, covering the
            # HAM clock ramp while the first input and weights stream in.
            wu_src = singles.tile([128, 512], mybir.dt.bfloat16)
            nc.vector.memset(wu_src[:], 0.0)
            wu = psum_pool.tile([128, NROW, 512], mybir.dt.float32, name="ps")
            for _ in range(WU_ROUNDS):
                for cg in range(NCOL):
                    for r in range(NROW):
                        nc.tensor.matmul(
                            wu[64 * cg : 64 * cg + 64, r, : STRIP * W],
                            lhsT=wu_src[64 * r : 64 * r + 64, :64],
                            rhs=wu_src[64 * r : 64 * r + 64, :448],
                            start=True,
                            stop=True,
                            tile_position=(64 * r, 64 * cg),
                        )

            w_sb = singles.tile([128, NCHUNK, NTAP, 64], mybir.dt.bfloat16)
            nc.sync.dma_start(out=w_sb[:], in_=wpk[:])

            # input tiles, issued in consumption order (ck-major, img minor);
            # bufs=3 throttles SW-DGE so early tiles aren't starved by
            # round-robin across many queued DMAs. First tile split at the
            # row needed by batch 0 (strips 0-1 -> padded rows 0:18), first
            # part on HW-DGE for a fast path to the first matmul.
            SPLIT = NCOL * STRIP + 2  # 18
            xpads = {}
            for ck in range(NCHUNK):
                for img in range(BPC):
                    xp = xpad_pool.tile([128, HP, WP], mybir.dt.bfloat16, name="xpad")
                    src = xs[img, ck * 128 : (ck + 1) * 128]
                    if (ck, img) == (0, 0):
                        nc.sync.dma_start(out=xp[:, :SPLIT, :], in_=src[:, :SPLIT, :])
                        nc.gpsimd.dma_start(out=xp[:, SPLIT:, :], in_=src[:, SPLIT:, :])
                    else:
                        nc.gpsimd.dma_start(out=xp[:], in_=src)
                    xpads[(ck, img)] = xp

            # 56 instances = 2 chunks x (4 images x 7 strips); batches of 2
            n_batch = 0
            for ck in range(NCHUNK):
                insts = [(j // NSTRIP, j % NSTRIP) for j in range(BPC * NSTRIP)]
                for k in range(len(insts) // NCOL):
                    pair = insts[NCOL * k : NCOL * k + NCOL]
                    ps = psum_pool.tile([128, NROW, 512], mybir.dt.float32, name="ps")
                    for t in range(NTAP):
                        th, tw = divmod(t, KK)
                        for cg, (img, s) in enumerate(pair):
                            for r in range(NROW):
                                nc.tensor.matmul(
                                    ps[64 * cg : 64 * cg + 64, r, : STRIP * W],
                                    lhsT=w_sb[64 * r : 64 * r + 64, ck, t, :],
                                    rhs=xpads[(ck, img)][
                                        64 * r : 64 * r + 64,
                                        s * STRIP + th : s * STRIP + th + STRIP,
                                        tw : tw + W,
                                    ],
                                    start=(t == 0),
                                    stop=(t == NTAP - 1),
                                    tile_position=(64 * r, 64 * cg),
                                )
                    ot = ot_pool.tile(
                        [128, NROW, STRIP, W], mybir.dt.bfloat16, name="ot"
                    )
                    copy = nc.scalar.copy if n_batch % 2 == 0 else nc.vector.tensor_copy
                    copy(out=ot[:], in_=ps[:, :, : STRIP * W])
                    for cg, (img, s) in enumerate(pair):
                        dst = out[
                            img, ck * 128 : (ck + 1) * 128, s * STRIP : (s + 1) * STRIP, :
                        ].rearrange("(r p) h w -> p r h w", r=NROW)
                        nc.sync.dma_start(
                            out=dst, in_=ot[64 * cg : 64 * cg + 64]
                        )
                    n_batch += 1
    nc.finalize()
    return nc


_CACHE = {}


def kernel(x, w, trace=False):
    from concourse.bass_utils import run_bass_kernel_spmd

    x = np.asarray(x)
    w = np.ascontiguousarray(np.asarray(w), dtype=np.float32)

    if "nc" not in _CACHE:
        _CACHE["nc"] = _build_bass()
    nc = _CACHE["nc"]

    xbf = np.zeros((B, C, HP, WP), dtype=ml_dtypes.bfloat16)
    xbf[:, :, 1 : H + 1, 1 : W + 1] = x.astype(ml_dtypes.bfloat16)
    wpk = _pack_weights(w)
    in_maps = [
        {"xs": np.ascontiguousarray(xbf[i * BPC : (i + 1) * BPC]), "wpk": wpk}
        for i in range(N_CORES)
    ]
    res = run_bass_kernel_spmd(
        nc, in_maps, core_ids=list(range(N_CORES)), trace=trace
    )
    outs = np.concatenate([res.results[i]["out"] for i in range(N_CORES)], axis=0)
    if trace:
        kernel.last_result = res
    return outs.astype(np.float32)
